# revision 1
# baseline (speedup 1.0000x reference)
"""2-layer GCN encoder (PyG GCNConv semantics) on 8 Trainium2 NeuronCores.

Strategy (dst-sharded graph parallel, v2):
- Nodes are permuted (degree-sorted deal across the 8 cores, then (d0,d1)-lex
  within each core with a d1 re-sort inside bands of 4 tiles) and dst-sharded:
  core c owns rows [c*6272,(c+1)*6272) of the permuted node table (6250 real
  rows + 22 zero spares per core).
- GCN normalization is separable: norm(e) = dinv[src]*dinv[dst]. dinv[src] is
  folded into featT on the host for layer 1 and into the z1 epilogue for
  layer 2, so the AllGathered per-layer table g already carries the source
  normalization. Each layer: compute g per shard, AllGather g into a full
  table in DRAM (Shared), then gather source rows per edge slot with
  gpsimd.dma_gather and segment-reduce them on the vector engine.
- Gather calls are large (several tiles per call, up to ~4K tokens), use
  single_packet=False (the 64-desc/engine packet cap otherwise limits calls
  to 1024 tokens) and round-robin over 4 SWDGE queues, which runs Q7
  descriptor generation for up to 4 calls concurrently (~4x the single-queue
  rate that bottlenecked the v1 kernel).
- Slots are padded per tile to the max (d0,d1) in-degree pair; low/high table
  halves keep gather indices within int16.
- Host does index/layout preprocessing only; all FLOPs and feature movement
  run on device.
"""
import sys
import os

for _p in ("/opt/trn_rl_repo", "/root/.axon_site/_ro/trn_rl_repo"):
    if os.path.isdir(_p) and _p not in sys.path:
        sys.path.insert(0, _p)

import numpy as np
import concourse.bass as bass
import concourse.bacc as bacc
import concourse.tile as tile
import concourse.mybir as mybir
from concourse.masks import make_identity
from concourse.bass_utils import run_bass_kernel_spmd

F32 = mybir.dt.float32
I16 = mybir.dt.int16

N_NODES = 50000
IN_DIM = 256
OUT_DIM = 64
N_CORES = 8
TILES = 49                  # ceil(6250/128)
SH = TILES * 128            # 6272 rows per core shard (incl. 22 zero spares)
N_LOW = 5                   # cores 0..4 are the "low" table half
SPLIT = N_LOW * SH          # 31360 < 32768 (int16 gather index limit)
BAND = 4                    # d1 re-sort band, in tiles
CAP_S = 30                  # max slot-columns per gather call (<=3840 tokens)
MSG_BUFS = 4
N_QUEUES = 4


def _host_prep(feat, W1, b1, W2, b2, edge_index):
    N, C, T = N_NODES, N_CORES, TILES
    src0 = np.asarray(edge_index[0], dtype=np.int64)
    dst0 = np.asarray(edge_index[1], dtype=np.int64)
    loops = np.arange(N, dtype=np.int64)
    src = np.concatenate([src0, loops])
    dst = np.concatenate([dst0, loops])
    deg = np.bincount(dst, minlength=N).astype(np.int64)

    # deal degree-sorted nodes across cores (balances per-core edge counts
    # and aligns tile degree profiles across cores)
    order0 = np.argsort(deg, kind="stable")
    core = np.empty(N, np.int64)
    core[order0] = np.arange(N) % C

    # per-dst split degrees by source half
    is_low = core[src] < N_LOW
    d0 = np.bincount(dst[is_low], minlength=N)
    d1 = deg - d0

    # (d0, d1)-lex order within each core, then re-sort by d1 inside bands of
    # BAND tiles (cuts per-tile max-d1 padding) -> local slot j
    j = np.empty(N, np.int64)
    bs = BAND * 128
    for c in range(C):
        nodes_c = np.where(core == c)[0]
        o = nodes_c[np.lexsort((d1[nodes_c], d0[nodes_c]))]
        o2 = o.copy()
        for s in range(0, len(o), bs):
            seg = o[s:s + bs]
            o2[s:s + bs] = seg[np.argsort(d1[seg], kind="stable")]
        j[o2] = np.arange(len(o2))
    row = core * SH + j

    # per-tile slot maxes (shared across cores)
    S0 = np.zeros(T, np.int64)
    S1 = np.zeros(T, np.int64)
    tl = j // 128
    for t in range(T):
        m = tl == t
        if m.any():
            S0[t] = d0[m].max()
            S1[t] = d1[m].max()

    # edge -> slot assignment (per dst, low edges then high)
    e_order = np.argsort(row[dst] * 2 + (~is_low).astype(np.int64), kind="stable")
    es, ed, el = src[e_order], dst[e_order], is_low[e_order]
    key = row[ed] * 2 + (~el).astype(np.int64)
    occ = np.zeros(len(es), np.int64)
    _, first_idx, counts = np.unique(key, return_index=True, return_counts=True)
    for fi, cnt in zip(first_idx, counts):
        occ[fi:fi + cnt] = np.arange(cnt)

    n_real = N // C  # 6250; local rows >= n_real are zero spares (pad targets)
    iA = np.full((C, T, 128, max(1, int(S0.max()))), n_real, np.int64)
    iB = np.full((C, T, 128, max(1, int(S1.max()))), n_real, np.int64)
    ec = row[ed] // SH
    ep = (row[ed] % SH) % 128
    et = (row[ed] % SH) // 128
    lm = el
    iA[ec[lm], et[lm], ep[lm], occ[lm]] = row[es[lm]]
    hm = ~el
    iB[ec[hm], et[hm], ep[hm], occ[hm]] = row[es[hm]] - SPLIT

    def wrap16(v):
        # idx position j -> [j%16, j//16], replicated across the 8 Q7 cores
        w = v.reshape(-1, 16).T.astype(np.int16)
        return np.tile(w, (8, 1))

    percore_idx = []
    for c in range(C):
        colsA, colsB = [], []
        for t in range(T):
            if S0[t] > 0:
                colsA.append(iA[c, t, :, :S0[t]].T.reshape(-1))
            if S1[t] > 0:
                colsB.append(iB[c, t, :, :S1[t]].T.reshape(-1))
        vA = np.concatenate(colsA) if colsA else np.zeros(16, np.int64)
        vB = np.concatenate(colsB) if colsB else np.zeros(16, np.int64)
        percore_idx.append((wrap16(vA), wrap16(vB)))

    # dinv folded into featT (layer-1 source scaling)
    deg_f = deg.astype(np.float64)
    dinv = np.where(deg_f > 0, 1.0 / np.sqrt(deg_f), 0.0).astype(np.float32)

    featT = np.zeros((C, IN_DIM, SH), np.float32)
    dinvt = np.zeros((C, 128, T), np.float32)
    feat = np.asarray(feat, np.float32)
    for c in range(C):
        nodes_c = np.where(core == c)[0]
        featT[c][:, j[nodes_c]] = (feat[nodes_c] * dinv[nodes_c, None]).T
        dinvt[c, j[nodes_c] % 128, j[nodes_c] // 128] = dinv[nodes_c]

    W1 = np.asarray(W1, np.float32)
    W2 = np.asarray(W2, np.float32)
    b1 = np.asarray(b1, np.float32)
    b2 = np.asarray(b2, np.float32)
    use_bias = bool(np.any(b1 != 0) or np.any(b2 != 0))
    in_maps = []
    for c in range(C):
        in_maps.append({
            "featT": featT[c],
            "idxA": np.ascontiguousarray(percore_idx[c][0]),
            "idxB": np.ascontiguousarray(percore_idx[c][1]),
            "dinvt": dinvt[c],
            "dinv2t": dinvt[c] * dinvt[c],
            "W1": W1.reshape(2, 128, OUT_DIM),
            "W2": W2,
            "b1": np.broadcast_to(b1, (128, OUT_DIM)).copy(),
            "b2": np.broadcast_to(b2, (128, OUT_DIM)).copy(),
        })
    post = {"core": core, "j": j}
    return in_maps, S0.astype(int), S1.astype(int), use_bias, post


def _make_groups(S0, S1):
    """Greedy consecutive-tile groups with sum(S0)<=CAP_S and sum(S1)<=CAP_S."""
    groups = []
    cur = []
    a = b = 0
    for t in range(TILES):
        if cur and (a + S0[t] > CAP_S or b + S1[t] > CAP_S):
            groups.append(cur)
            cur = []
            a = b = 0
        cur.append(t)
        a += S0[t]
        b += S1[t]
    if cur:
        groups.append(cur)
    return groups


def _build_nc(S0, S1, use_bias=False, reps=1):
    C, T, D = N_CORES, TILES, OUT_DIM
    KIN = IN_DIM // 128
    CA = int(sum(S0)) * 8
    CB = int(sum(S1)) * 8
    groups = _make_groups(S0, S1)
    msgA_cols = max(int(sum(S0[t] for t in g)) for g in groups)
    msgB_cols = max(int(sum(S1[t] for t in g)) for g in groups)
    nc = bacc.Bacc(None, target_bir_lowering=False, num_swdge_queues=N_QUEUES)
    featT = nc.dram_tensor("featT", [IN_DIM, SH], F32, kind="ExternalInput")
    idxA = nc.dram_tensor("idxA", [128, max(CA, 16)], I16, kind="ExternalInput")
    idxB = nc.dram_tensor("idxB", [128, max(CB, 16)], I16, kind="ExternalInput")
    dinvt = nc.dram_tensor("dinvt", [128, T], F32, kind="ExternalInput")
    dinv2t = nc.dram_tensor("dinv2t", [128, T], F32, kind="ExternalInput")
    W1 = nc.dram_tensor("W1", [KIN, 128, D], F32, kind="ExternalInput")
    W2 = nc.dram_tensor("W2", [D, D], F32, kind="ExternalInput")
    b1 = nc.dram_tensor("b1", [128, D], F32, kind="ExternalInput")
    b2 = nc.dram_tensor("b2", [128, D], F32, kind="ExternalInput")
    out = nc.dram_tensor("out", [SH, D], F32, kind="ExternalOutput")

    with tile.TileContext(nc) as tc:
        with (
            tc.tile_pool(name="dram", bufs=1, space="DRAM") as dramp,
            tc.tile_pool(name="const", bufs=1) as constp,
            tc.tile_pool(name="feat", bufs=1) as featp,
            tc.tile_pool(name="gz", bufs=4) as gzp,
            tc.tile_pool(name="msga", bufs=5) as msgap,
            tc.tile_pool(name="msgb", bufs=5) as msgbp,
            tc.tile_pool(name="ps", bufs=4, space="PSUM") as psp,
        ):
            fts = []
            for k in range(KIN):
                ftk = featp.tile([128, SH], F32, name=f"ft{k}")
                nc.sync.dma_start(out=ftk[:], in_=featT[k * 128:(k + 1) * 128, :])
                fts.append(ftk)
            w1s = []
            for k in range(KIN):
                w1k = constp.tile([128, D], F32, name=f"w1{k}")
                nc.sync.dma_start(out=w1k[:], in_=W1[k, :, :])
                w1s.append(w1k)
            w2 = constp.tile([D, D], F32)
            nc.sync.dma_start(out=w2[:], in_=W2[:, :])
            b1t = constp.tile([128, D], F32)
            nc.sync.dma_start(out=b1t[:], in_=b1[:, :])
            b2t = constp.tile([128, D], F32)
            nc.sync.dma_start(out=b2t[:], in_=b2[:, :])
            ia = constp.tile([128, max(CA, 16)], I16)
            nc.sync.dma_start(out=ia[:], in_=idxA[:, :])
            ib = constp.tile([128, max(CB, 16)], I16)
            nc.sync.dma_start(out=ib[:], in_=idxB[:, :])
            dinv = constp.tile([128, T], F32)
            nc.sync.dma_start(out=dinv[:], in_=dinvt[:, :])
            dinv2 = constp.tile([128, T], F32)
            nc.sync.dma_start(out=dinv2[:], in_=dinv2t[:, :])
            ident = constp.tile([128, 128], F32)
            make_identity(nc, ident[:])
            z1T = constp.tile([D, SH], F32)

            ag_in = [dramp.tile([SH, D], F32, name=f"agin{l}") for l in range(2)]

            qn = [0]
            last_gather = [None]
            RELU = mybir.ActivationFunctionType.Relu
            COPY = mybir.ActivationFunctionType.Copy

            def chain(inst):
                # Pin Pool-engine emission order of gathers: the Tile
                # scheduler assigns DMASW sem lanes round-robin in scheduled
                # order and each lane is locked to one SWDGE queue, so the
                # scheduled order must match the queue_num rotation.
                if last_gather[0] is not None:
                    inst.ins.add_dependency(last_gather[0].ins.name,
                                            mybir.DependencyInfo.NO_SYNC_ONLY)
                last_gather[0] = inst

            def tree(msgt, off, S):
                # in-place pairwise tree; leaves the sum at block `off`
                Wc = S
                while Wc > 1:
                    h = Wc // 2
                    nc.vector.tensor_add(
                        msgt[:, off * D:(off + h) * D],
                        msgt[:, off * D:(off + h) * D],
                        msgt[:, (off + Wc - h) * D:(off + Wc) * D])
                    Wc -= h

            def aggregate(tbl, bias, is_last):
                sc = dinv if is_last else dinv2
                colA = 0
                colB = 0
                for grp in groups:
                    gS0 = int(sum(S0[t] for t in grp))
                    gS1 = int(sum(S1[t] for t in grp))
                    msgA = msgap.tile([128, msgA_cols * D], F32, tag="msga")
                    msgB = msgbp.tile([128, msgB_cols * D], F32, tag="msgb")
                    if gS0 > 0:
                        n = gS0 * 128
                        chain(nc.gpsimd.dma_gather(
                            msgA[:, :gS0 * D].rearrange("p (s d) -> p s d", d=D),
                            tbl[:SPLIT, :],
                            ia[:, colA:colA + gS0 * 8],
                            n, n, D, elem_step=D,
                            single_packet=False,
                            queue_num=qn[0] % N_QUEUES))
                        qn[0] += 1
                        colA += gS0 * 8
                    if gS1 > 0:
                        n = gS1 * 128
                        chain(nc.gpsimd.dma_gather(
                            msgB[:, :gS1 * D].rearrange("p (s d) -> p s d", d=D),
                            tbl[SPLIT:, :],
                            ib[:, colB:colB + gS1 * 8],
                            n, n, D, elem_step=D,
                            single_packet=False,
                            queue_num=qn[0] % N_QUEUES))
                        qn[0] += 1
                        colB += gS1 * 8
                    offA = 0
                    offB = 0
                    for t in grp:
                        s0, s1 = int(S0[t]), int(S1[t])
                        tree(msgA, offA, s0)
                        tree(msgB, offB, s1)
                        if s0 > 0:
                            acc = msgA[:, offA * D:(offA + 1) * D]
                            if s1 > 0:
                                nc.vector.tensor_add(
                                    acc, acc, msgB[:, offB * D:(offB + 1) * D])
                        else:
                            acc = msgB[:, offB * D:(offB + 1) * D]
                        offA += s0
                        offB += s1
                        zt = gzp.tile([128, D], F32, tag="z")
                        if use_bias:
                            # z = relu(acc*dinv + b); layer-1 additionally *dinv
                            nc.vector.scalar_tensor_tensor(
                                zt[:], acc, dinv[:, t:t + 1], bias[:],
                                op0=mybir.AluOpType.mult, op1=mybir.AluOpType.add)
                            nc.scalar.activation(zt[:], zt[:], RELU)
                            if not is_last:
                                nc.vector.scalar_tensor_tensor(
                                    zt[:], zt[:], dinv[:, t:t + 1], zt[:],
                                    op0=mybir.AluOpType.mult,
                                    op1=mybir.AluOpType.bypass)
                        else:
                            # b == 0: relu(acc*dinv)[*dinv] == relu(acc*scale),
                            # scale = dinv (last layer) or dinv^2 (layer 1)
                            nc.scalar.activation(zt[:], acc, RELU,
                                                 scale=sc[:, t:t + 1])
                        if is_last:
                            nc.sync.dma_start(
                                out=out[t * 128:(t + 1) * 128, :], in_=zt[:])
                        else:
                            pst = psp.tile([D, 128], F32, tag="tr")
                            nc.tensor.transpose(out=pst[:], in_=zt[:],
                                                identity=ident[:])
                            nc.scalar.activation(
                                z1T[:, t * 128:(t + 1) * 128], pst[:], COPY)

            for rep in range(reps):
                # Shared DRAM tensors are single-writer: fresh tables per rep
                table = [dramp.tile([C * SH, D], F32, name=f"table{l}_r{rep}",
                                    addr_space="Shared") for l in range(2)]
                # layer-1 g: (feat*dinv) @ W1   (dinv pre-folded on host)
                for t in range(T):
                    ps = psp.tile([128, D], F32, tag="mm")
                    for k in range(KIN):
                        nc.tensor.matmul(ps[:], lhsT=fts[k][:, t * 128:(t + 1) * 128],
                                         rhs=w1s[k][:, :],
                                         start=(k == 0), stop=(k == KIN - 1))
                    g = gzp.tile([128, D], F32, tag="g")
                    nc.scalar.activation(g[:], ps[:], COPY)
                    nc.sync.dma_start(out=ag_in[0][t * 128:(t + 1) * 128, :], in_=g[:])
                for layer in range(2):
                    nc.gpsimd.collective_compute(
                        "AllGather", mybir.AluOpType.bypass,
                        replica_groups=[list(range(C))],
                        ins=[ag_in[layer][:]],
                        outs=[table[layer][:]],
                    )
                    if layer == 0:
                        aggregate(table[0], b1t, is_last=False)
                        # layer-2 g: (z1*dinv) @ W2 (dinv applied in epilogue)
                        for t in range(T):
                            ps = psp.tile([128, D], F32, tag="mm")
                            nc.tensor.matmul(ps[:], lhsT=z1T[:, t * 128:(t + 1) * 128],
                                             rhs=w2[:, :], start=True, stop=True)
                            g = gzp.tile([128, D], F32, tag="g")
                            nc.scalar.activation(g[:], ps[:], COPY)
                            nc.sync.dma_start(
                                out=ag_in[1][t * 128:(t + 1) * 128, :], in_=g[:])
                    else:
                        aggregate(table[1], b2t, is_last=True)

    nc.finalize()
    return nc


def kernel(feat, W1, b1, W2, b2, edge_index, _reps=1, _return_nc=False):
    in_maps, S0, S1, use_bias, post = _host_prep(feat, W1, b1, W2, b2, edge_index)
    nc = _build_nc(S0, S1, use_bias=use_bias, reps=_reps)
    if _return_nc:
        return nc, in_maps, post
    res = run_bass_kernel_spmd(nc, in_maps, core_ids=list(range(N_CORES)))
    full = np.empty((N_NODES, OUT_DIM), np.float32)
    core, j = post["core"], post["j"]
    for c in range(N_CORES):
        oc = res.results[c]["out"]
        nodes_c = np.where(core == c)[0]
        full[nodes_c] = oc[j[nodes_c]]
    return full



# revision 7
# speedup vs baseline: 1.4890x; 1.4890x over previous
"""2-layer GCN encoder (PyG GCNConv semantics) on 8 Trainium2 NeuronCores.

Strategy (dst-sharded graph parallel, v4):
- Nodes are permuted (degree-sorted deal across the 8 cores) and dst-sharded:
  core c owns 6250 real rows + 22 zero spares (SH=6272 = 49 tiles of 128).
- Within each core, nodes are split into two chunk-halves (3179 real + 21
  spares in chunk 0 = tiles 0-24; 3071 real + 1 spare in chunk 1 = tiles
  25-48). The per-layer gather table is TWO Shared DRAM tensors (chunk 0:
  8x3200 rows, chunk 1: 8x3072 rows), each filled by its own AllGather, so
  each tensor has a single writer and each gather call's int16 index window
  covers one whole tensor (25600 / 24576 < 32768).
- Every edge is gathered from the chunk its source lives in (call A = chunk 0,
  call B = chunk 1). Which half a node lives in is chosen by a sequential
  greedy discrepancy balancer (minimize sum over dsts of (nA-nB)^2) so each
  dst's in-edges split nearly evenly, which with a (nlo, nhi)-lex tile sort
  (+ banded re-sort) nearly eliminates per-tile slot padding.
- Self-loop messages never go through the gather: the local pre-AllGather g
  tile is added in the epilogue instead.
- GCN normalization is separable: dinv[src] is folded into featT on the host
  (layer 1) and into the z1 epilogue scale (layer 2); dinv[dst] is the
  epilogue scale.
- Gather descriptor generation on the 4 SWDGE queues is the throughput limit
  (~2.2-2.7ns/token with all 4 queues streaming); calls rotate queues in
  emission order (the Tile scheduler assigns DMASW sem lanes round-robin in
  scheduled order and each lane is locked to one queue, so scheduled order
  must match the rotation) and msg pools are deep enough to keep 4+ groups
  in flight.
- Layer-2 work is pipelined per tile (transpose + W2 matmul immediately after
  each layer-1 tile's epilogue) and each AllGather chunk launches as soon as
  its half of the shard is ready, hiding most of the collective time under
  the previous gather phase.
- Host does index/layout preprocessing only; all FLOPs and feature movement
  run on device.
"""
import sys
import os

for _p in ("/opt/trn_rl_repo", "/root/.axon_site/_ro/trn_rl_repo"):
    if os.path.isdir(_p) and _p not in sys.path:
        sys.path.insert(0, _p)

import numpy as np
import concourse.bass as bass
import concourse.bacc as bacc
import concourse.tile as tile
import concourse.mybir as mybir
from concourse.masks import make_identity
from concourse.bass_utils import run_bass_kernel_spmd

F32 = mybir.dt.float32
I16 = mybir.dt.int16

N_NODES = 50000
IN_DIM = 256
OUT_DIM = 64
N_CORES = 8
TILES = 49                  # ceil(6250/128)
SH = TILES * 128            # 6272 rows per core shard (incl. 22 zero spares)
N_REAL = N_NODES // N_CORES  # 6250
CH0 = 3200                  # chunk-0 local rows (25 tiles); chunk 1 has 3072
CH1 = SH - CH0              # 3072
CH0_TILES = 25
C0_REAL = 3179              # real nodes in chunk 0 (21 spares)
C1_REAL = 3071              # real nodes in chunk 1 (1 spare)
PAD_A = C0_REAL             # core-0 chunk-0 spare row (always zero)
PAD_B = C1_REAL             # core-0 chunk-1 spare row (always zero)
CAP_S = 24                  # max slot-columns per gather call (<=3072 tokens)
MSG_BUFS = 6
N_QUEUES = 4
BAND = 4


def _balance_sides(src, dst, core, deg):
    """Assign each node a chunk-half so every dst's in-edges split evenly.

    Sequential greedy (high out-degree first) + local-flip refinement on
    sum(imb^2), then exact per-core cardinality repair. Returns side[N] in
    {+1 (chunk 0), -1 (chunk 1)}.
    """
    N = len(deg)
    o_src = np.argsort(src, kind="stable")
    dst_by_src = dst[o_src]
    outdeg = np.bincount(src, minlength=N)
    ptr = np.zeros(N + 1, np.int64)
    ptr[1:] = np.cumsum(outdeg)

    side = np.zeros(N, np.int64)
    imb = np.zeros(N, np.int64)
    for v in np.argsort(-outdeg, kind="stable"):
        ds = dst_by_src[ptr[v]:ptr[v + 1]]
        s = -1 if imb[ds].sum() > 0 else 1
        side[v] = s
        imb[ds] += s
    for _ in range(3):
        nf = 0
        for v in range(N):
            ds = dst_by_src[ptr[v]:ptr[v + 1]]
            s = side[v]
            if -4 * s * imb[ds].sum() + 4 * len(ds) < 0:
                side[v] = -s
                imb[ds] -= 2 * s
                nf += 1
        if nf == 0:
            break
    for c in range(N_CORES):
        nodes_c = np.where(core == c)[0]
        need = int((side[nodes_c] == 1).sum()) - C0_REAL
        if need == 0:
            continue
        sgn = 1 if need > 0 else -1
        cand = nodes_c[side[nodes_c] == sgn]
        gains = np.array([-4 * sgn * imb[dst_by_src[ptr[v]:ptr[v + 1]]].sum()
                          + 4 * (ptr[v + 1] - ptr[v]) for v in cand])
        for v in cand[np.argsort(gains)][:abs(need)]:
            ds = dst_by_src[ptr[v]:ptr[v + 1]]
            imb[ds] -= 2 * side[v]
            side[v] = -side[v]
    return side


def _host_prep(feat, W1, b1, W2, b2, edge_index):
    N, C, T = N_NODES, N_CORES, TILES
    src = np.asarray(edge_index[0], dtype=np.int64)
    dst = np.asarray(edge_index[1], dtype=np.int64)
    E = src.shape[0]
    deg_full = np.bincount(dst, minlength=N) + 1  # incl. the explicit self-loop
    deg = deg_full - 1                            # gathered (loopless) in-degree

    # deal degree-sorted nodes across cores (balances per-core edge counts
    # and aligns tile degree profiles across cores)
    order0 = np.argsort(deg, kind="stable")
    core = np.empty(N, np.int64)
    core[order0] = np.arange(N) % C

    side = _balance_sides(src, dst, core, deg)
    e_half = (side[src] < 0).astype(np.int64)  # 0 = A/chunk0, 1 = B/chunk1
    nlo = np.bincount(dst[e_half == 0], minlength=N)
    nhi = deg - nlo

    # within-core, within-half order: lex (nlo, nhi) with banded re-sort on
    # nhi (cuts per-tile max-nhi padding); spares sit at the end of each half
    j = np.empty(N, np.int64)
    for c in range(C):
        for sgn, base in ((1, 0), (-1, CH0)):
            nodes = np.where((core == c) & (side == sgn))[0]
            o = nodes[np.lexsort((nhi[nodes], nlo[nodes]))]
            o2 = o.copy()
            bs = BAND * 128
            for s in range(0, len(o), bs):
                seg = o[s:s + bs]
                o2[s:s + bs] = seg[np.argsort(nhi[seg], kind="stable")]
            j[o2] = base + np.arange(len(o2))

    # per-tile slot maxima, shared across cores (SPMD program)
    S0g = np.zeros(T, np.int64)
    S1g = np.zeros(T, np.int64)
    for c in range(C):
        nodes_c = np.where(core == c)[0]
        jj = j[nodes_c]
        for t in range(T):
            nodes = nodes_c[(jj >= t * 128) & (jj < (t + 1) * 128)]
            if len(nodes) == 0:
                continue
            S0g[t] = max(S0g[t], int(nlo[nodes].max()))
            S1g[t] = max(S1g[t], int(nhi[nodes].max()))

    # slot occupancy per (dst, half)
    keys = j[dst] + SH * core[dst]
    e_order = np.argsort(keys * 2 + e_half, kind="stable")
    es, ed, eh = src[e_order], dst[e_order], e_half[e_order]
    key2 = keys[e_order] * 2 + eh
    _, first_idx, counts = np.unique(key2, return_index=True, return_counts=True)
    occ = np.arange(E) - np.repeat(first_idx, counts)

    # chunk-local gather indices
    idx_in_chunk = np.where(j < CH0, core * CH0 + j,
                            core * CH1 + (j - CH0))

    iA = np.full((C, T, 128, max(1, int(S0g.max()))), PAD_A, np.int64)
    iB = np.full((C, T, 128, max(1, int(S1g.max()))), PAD_B, np.int64)
    ec = core[ed]
    ep = j[ed] % 128
    et = j[ed] // 128
    am = eh == 0
    iA[ec[am], et[am], ep[am], occ[am]] = idx_in_chunk[es[am]]
    bm = eh == 1
    iB[ec[bm], et[bm], ep[bm], occ[bm]] = idx_in_chunk[es[bm]]
    assert iA.max() < C * CH0 and iA.min() >= 0
    assert iB.max() < C * CH1 and iB.min() >= 0

    def wrap16(v):
        # idx position j -> [j%16, j//16], replicated across the 8 Q7 cores
        w = v.reshape(-1, 16).T.astype(np.int16)
        return np.tile(w, (8, 1))

    percore_idx = []
    for c in range(C):
        colsA, colsB = [], []
        for t in range(T):
            if S0g[t] > 0:
                colsA.append(iA[c, t, :, :S0g[t]].T.reshape(-1))
            if S1g[t] > 0:
                colsB.append(iB[c, t, :, :S1g[t]].T.reshape(-1))
        vA = np.concatenate(colsA) if colsA else np.zeros(16, np.int64)
        vB = np.concatenate(colsB) if colsB else np.zeros(16, np.int64)
        percore_idx.append((wrap16(vA), wrap16(vB)))

    # dinv folded into featT (layer-1 source scaling)
    deg_f = deg_full.astype(np.float64)
    dinv = (1.0 / np.sqrt(deg_f)).astype(np.float32)

    featT = np.zeros((C, IN_DIM, SH), np.float32)
    dinvt = np.zeros((C, 128, T), np.float32)
    feat = np.asarray(feat, np.float32)
    for c in range(C):
        nodes_c = np.where(core == c)[0]
        featT[c][:, j[nodes_c]] = (feat[nodes_c] * dinv[nodes_c, None]).T
        dinvt[c, j[nodes_c] % 128, j[nodes_c] // 128] = dinv[nodes_c]

    W1 = np.asarray(W1, np.float32)
    W2 = np.asarray(W2, np.float32)
    b1 = np.asarray(b1, np.float32)
    b2 = np.asarray(b2, np.float32)
    use_bias = bool(np.any(b1 != 0) or np.any(b2 != 0))
    in_maps = []
    for c in range(C):
        in_maps.append({
            "featT": featT[c],
            "idxA": np.ascontiguousarray(percore_idx[c][0]),
            "idxB": np.ascontiguousarray(percore_idx[c][1]),
            "dinvt": dinvt[c],
            "dinv2t": dinvt[c] * dinvt[c],
            "W1": W1.reshape(2, 128, OUT_DIM),
            "W2": W2,
            "b1": np.broadcast_to(b1, (128, OUT_DIM)).copy(),
            "b2": np.broadcast_to(b2, (128, OUT_DIM)).copy(),
        })
    post = {"core": core, "j": j}
    return in_maps, S0g.astype(int), S1g.astype(int), use_bias, post


def _make_groups(S0, S1):
    """Greedy consecutive-tile groups, sum(S0)<=CAP_S and sum(S1)<=CAP_S,
    with a forced break at the chunk boundary (tile CH0_TILES)."""
    groups = []
    cur = []
    a = b = 0
    for t in range(TILES):
        if cur and (a + S0[t] > CAP_S or b + S1[t] > CAP_S or t == CH0_TILES):
            groups.append(cur)
            cur = []
            a = b = 0
        cur.append(t)
        a += S0[t]
        b += S1[t]
    if cur:
        groups.append(cur)
    return groups


def _build_nc(S0, S1, use_bias=False, reps=1):
    C, T, D = N_CORES, TILES, OUT_DIM
    KIN = IN_DIM // 128
    CA = int(sum(S0)) * 8
    CB = int(sum(S1)) * 8
    groups = _make_groups(S0, S1)
    msgA_cols = max(int(sum(S0[t] for t in g)) for g in groups)
    msgB_cols = max(int(sum(S1[t] for t in g)) for g in groups)
    nc = bacc.Bacc(None, target_bir_lowering=False, num_swdge_queues=N_QUEUES)
    featT = nc.dram_tensor("featT", [IN_DIM, SH], F32, kind="ExternalInput")
    idxA = nc.dram_tensor("idxA", [128, max(CA, 16)], I16, kind="ExternalInput")
    idxB = nc.dram_tensor("idxB", [128, max(CB, 16)], I16, kind="ExternalInput")
    dinvt = nc.dram_tensor("dinvt", [128, T], F32, kind="ExternalInput")
    dinv2t = nc.dram_tensor("dinv2t", [128, T], F32, kind="ExternalInput")
    W1 = nc.dram_tensor("W1", [KIN, 128, D], F32, kind="ExternalInput")
    W2 = nc.dram_tensor("W2", [D, D], F32, kind="ExternalInput")
    b1 = nc.dram_tensor("b1", [128, D], F32, kind="ExternalInput")
    b2 = nc.dram_tensor("b2", [128, D], F32, kind="ExternalInput")
    out = nc.dram_tensor("out", [SH, D], F32, kind="ExternalOutput")

    with tile.TileContext(nc) as tc:
        with (
            tc.tile_pool(name="dram", bufs=1, space="DRAM") as dramp,
            tc.tile_pool(name="const", bufs=1) as constp,
            tc.tile_pool(name="feat", bufs=1) as featp,
            tc.tile_pool(name="gz", bufs=4) as gzp,
            tc.tile_pool(name="msga", bufs=MSG_BUFS) as msgap,
            tc.tile_pool(name="msgb", bufs=MSG_BUFS) as msgbp,
            tc.tile_pool(name="ps", bufs=4, space="PSUM") as psp,
        ):
            fts = []
            for k in range(KIN):
                ftk = featp.tile([128, SH], F32, name=f"ft{k}")
                nc.sync.dma_start(out=ftk[:], in_=featT[k * 128:(k + 1) * 128, :])
                fts.append(ftk)
            w1s = []
            for k in range(KIN):
                w1k = constp.tile([128, D], F32, name=f"w1{k}")
                nc.sync.dma_start(out=w1k[:], in_=W1[k, :, :])
                w1s.append(w1k)
            w2 = constp.tile([D, D], F32)
            nc.sync.dma_start(out=w2[:], in_=W2[:, :])
            b1t = constp.tile([128, D], F32)
            nc.sync.dma_start(out=b1t[:], in_=b1[:, :])
            b2t = constp.tile([128, D], F32)
            nc.sync.dma_start(out=b2t[:], in_=b2[:, :])
            ia = constp.tile([128, max(CA, 16)], I16)
            nc.sync.dma_start(out=ia[:], in_=idxA[:, :])
            ib = constp.tile([128, max(CB, 16)], I16)
            nc.sync.dma_start(out=ib[:], in_=idxB[:, :])
            dinv = constp.tile([128, T], F32)
            nc.sync.dma_start(out=dinv[:], in_=dinvt[:, :])
            dinv2 = constp.tile([128, T], F32)
            nc.sync.dma_start(out=dinv2[:], in_=dinv2t[:, :])
            ident = constp.tile([128, 128], F32)
            make_identity(nc, ident[:])
            z1T = constp.tile([D, SH], F32)
            gloc = [constp.tile([128, T * D], F32, name=f"gloc{l}")
                    for l in range(2)]

            ag_in = [dramp.tile([SH, D], F32, name=f"agin{l}") for l in range(2)]

            qn = [0]
            last_gather = [None]
            RELU = mybir.ActivationFunctionType.Relu
            COPY = mybir.ActivationFunctionType.Copy

            def chain(inst):
                # Pin Pool-engine emission order of gathers: the Tile
                # scheduler assigns DMASW sem lanes round-robin in scheduled
                # order and each lane is locked to one SWDGE queue, so the
                # scheduled order must match the queue_num rotation.
                if last_gather[0] is not None:
                    inst.ins.add_dependency(last_gather[0].ins.name,
                                            mybir.DependencyInfo.NO_SYNC_ONLY)
                last_gather[0] = inst

            def tree(msgt, off, S):
                # in-place pairwise tree; leaves the sum at block `off`
                Wc = S
                while Wc > 1:
                    h = Wc // 2
                    nc.vector.tensor_add(
                        msgt[:, off * D:(off + h) * D],
                        msgt[:, off * D:(off + h) * D],
                        msgt[:, (off + Wc - h) * D:(off + Wc) * D])
                    Wc -= h

            def emit_ag(layer, tbl, chunk):
                if chunk == 0:
                    nc.gpsimd.collective_compute(
                        "AllGather", mybir.AluOpType.bypass,
                        replica_groups=[list(range(C))],
                        ins=[ag_in[layer][0:CH0, :]],
                        outs=[tbl[:, :]])
                else:
                    nc.gpsimd.collective_compute(
                        "AllGather", mybir.AluOpType.bypass,
                        replica_groups=[list(range(C))],
                        ins=[ag_in[layer][CH0:SH, :]],
                        outs=[tbl[:, :]])

            def mm2_tile(t, zt):
                # z1'[t] is in zt; transpose, matmul W2, stash g2 and ship row
                pst = psp.tile([D, 128], F32, tag="tr", bufs=3)
                nc.tensor.transpose(out=pst[:], in_=zt[:], identity=ident[:])
                nc.scalar.activation(
                    z1T[:, t * 128:(t + 1) * 128], pst[:], COPY)
                ps2 = psp.tile([128, D], F32, tag="mm2", bufs=3)
                nc.tensor.matmul(ps2[:], lhsT=z1T[:, t * 128:(t + 1) * 128],
                                 rhs=w2[:, :], start=True, stop=True)
                nc.scalar.activation(gloc[1][:, t * D:(t + 1) * D], ps2[:], COPY)
                nc.sync.dma_start(out=ag_in[1][t * 128:(t + 1) * 128, :],
                                  in_=gloc[1][:, t * D:(t + 1) * D])

            def aggregate(tblA, tblB, bias, is_last, tables2=None):
                sc = dinv if is_last else dinv2
                colA = 0
                colB = 0
                done_tiles = 0
                for grp in groups:
                    gS0 = int(sum(S0[t] for t in grp))
                    gS1 = int(sum(S1[t] for t in grp))
                    msgA = msgap.tile([128, msgA_cols * D], F32, tag="msga")
                    msgB = msgbp.tile([128, msgB_cols * D], F32, tag="msgb")
                    if gS0 > 0:
                        n = gS0 * 128
                        chain(nc.gpsimd.dma_gather(
                            msgA[:, :gS0 * D].rearrange("p (s d) -> p s d", d=D),
                            tblA[:, :],
                            ia[:, colA:colA + gS0 * 8],
                            n, n, D, elem_step=D,
                            single_packet=False,
                            queue_num=qn[0] % N_QUEUES))
                        qn[0] += 1
                        colA += gS0 * 8
                    if gS1 > 0:
                        n = gS1 * 128
                        chain(nc.gpsimd.dma_gather(
                            msgB[:, :gS1 * D].rearrange("p (s d) -> p s d", d=D),
                            tblB[:, :],
                            ib[:, colB:colB + gS1 * 8],
                            n, n, D, elem_step=D,
                            single_packet=False,
                            queue_num=qn[0] % N_QUEUES))
                        qn[0] += 1
                        colB += gS1 * 8
                    offA = 0
                    offB = 0
                    for t in grp:
                        s0, s1 = int(S0[t]), int(S1[t])
                        tree(msgA, offA, s0)
                        tree(msgB, offB, s1)
                        if s0 > 0:
                            acc = msgA[:, offA * D:(offA + 1) * D]
                            if s1 > 0:
                                nc.vector.tensor_add(
                                    acc, acc, msgB[:, offB * D:(offB + 1) * D])
                        else:
                            acc = msgB[:, offB * D:(offB + 1) * D]
                        # self-loop message: local g tile
                        nc.vector.tensor_add(
                            acc, acc,
                            gloc[1 if is_last else 0][:, t * D:(t + 1) * D])
                        offA += s0
                        offB += s1
                        zt = gzp.tile([128, D], F32, tag="z")
                        if use_bias:
                            # z = relu(acc*dinv + b); layer-1 additionally *dinv
                            nc.vector.scalar_tensor_tensor(
                                zt[:], acc, dinv[:, t:t + 1], bias[:],
                                op0=mybir.AluOpType.mult, op1=mybir.AluOpType.add)
                            nc.scalar.activation(zt[:], zt[:], RELU)
                            if not is_last:
                                nc.vector.scalar_tensor_tensor(
                                    zt[:], zt[:], dinv[:, t:t + 1], zt[:],
                                    op0=mybir.AluOpType.mult,
                                    op1=mybir.AluOpType.bypass)
                        else:
                            # b == 0: relu(acc*dinv)[*dinv] == relu(acc*scale),
                            # scale = dinv (last layer) or dinv^2 (layer 1)
                            nc.scalar.activation(zt[:], acc, RELU,
                                                 scale=sc[:, t:t + 1])
                        if is_last:
                            nc.sync.dma_start(
                                out=out[t * 128:(t + 1) * 128, :], in_=zt[:])
                        else:
                            mm2_tile(t, zt)
                        done_tiles += 1
                    if not is_last and tables2 is not None:
                        if done_tiles == CH0_TILES:
                            emit_ag(1, tables2[0], 0)
                        elif done_tiles == T:
                            emit_ag(1, tables2[1], 1)

            for rep in range(reps):
                # Shared DRAM tensors are single-writer: fresh tables per rep,
                # one tensor per (layer, chunk)
                tbls = [[dramp.tile([C * CH0, D], F32, name=f"tbl{l}a_r{rep}",
                                    addr_space="Shared"),
                         dramp.tile([C * CH1, D], F32, name=f"tbl{l}b_r{rep}",
                                    addr_space="Shared")] for l in range(2)]
                # layer-1 g: (feat*dinv) @ W1   (dinv pre-folded on host)
                for t in range(T):
                    ps = psp.tile([128, D], F32, tag="mm", bufs=2)
                    for k in range(KIN):
                        nc.tensor.matmul(ps[:], lhsT=fts[k][:, t * 128:(t + 1) * 128],
                                         rhs=w1s[k][:, :],
                                         start=(k == 0), stop=(k == KIN - 1))
                    nc.scalar.activation(gloc[0][:, t * D:(t + 1) * D], ps[:], COPY)
                    nc.sync.dma_start(out=ag_in[0][t * 128:(t + 1) * 128, :],
                                      in_=gloc[0][:, t * D:(t + 1) * D])
                    if t == CH0_TILES - 1:
                        emit_ag(0, tbls[0][0], 0)
                emit_ag(0, tbls[0][1], 1)
                aggregate(tbls[0][0], tbls[0][1], b1t, is_last=False,
                          tables2=tbls[1])
                aggregate(tbls[1][0], tbls[1][1], b2t, is_last=True)

    nc.finalize()
    return nc


def kernel(feat, W1, b1, W2, b2, edge_index, _reps=1, _return_nc=False):
    in_maps, S0, S1, use_bias, post = _host_prep(feat, W1, b1, W2, b2, edge_index)
    nc = _build_nc(S0, S1, use_bias=use_bias, reps=_reps)
    if _return_nc:
        return nc, in_maps, post
    res = run_bass_kernel_spmd(nc, in_maps, core_ids=list(range(N_CORES)))
    full = np.empty((N_NODES, OUT_DIM), np.float32)
    core, j = post["core"], post["j"]
    for c in range(N_CORES):
        oc = res.results[c]["out"]
        nodes_c = np.where(core == c)[0]
        full[nodes_c] = oc[j[nodes_c]]
    return full
